# revision 17
# baseline (speedup 1.0000x reference)
"""Adaptive thresholding (11x11 box mean, BORDER_REPLICATE, THRESH_BINARY_INV)
on 8 TRN2 NeuronCores, data-parallel over the batch dim (16 images/core).

v7 design — fused-halo 118-row blocks:
  - Each image is split into 5 row-blocks of [118,118,118,118,40] rows. For
    block b the host packs a [128, 533] fp16 segment: partitions 0-117 the
    block rows, 118-122 the next 5 rows, 123-127 the previous 5 rows
    (BORDER_REPLICATE at the image edges), all scaled y = fp16(x)/4 with an
    11-col zero head and 5-col horizontal replicate margins. Vertical halos
    and edge handling thus live in the DATA, so ONE uniform band matrix
    bmh [128,118] computes the complete vertical sum per block: 2 matmuls
    per block (idn + bmh), 10 per image, 2 LDWEIGHTS groups, and no
    cross-segment PE dependencies.
  - Custom DVE op ADAPT_WSCAN: out = inclusive_scan_add(Src0 - Src1) at
    1 elem/cycle; one scan per image over the flat [128, 5*533] view; the
    duplicated halo partitions are scanned for free (partition-parallel).
  - Block4's dead partitions 40-117 are zeroed on-chip (gpsimd memset) and
    skipped by the DMA (the x plane for block4 is fetched as two partition
    slices).
  - PSUM: tile A [128,3,512] (blocks 0-2, 2 bufs) + tile B [128,2,512]
    (blocks 3-4, 1 buf) = exactly 8 banks. Per image: Sign(PSUM - 60.5)
    as two merged activations (B first so its single buffer recycles
    early) -> fp8e4m3, out at 1 byte/px.
Host: out = (sign >= 0) * 255  (inclusive compare matches x <= mean-2).
"""
import sys
sys.path.insert(0, '/opt/trn_rl_repo')
import numpy as np
import concourse.bass as bass
import concourse.tile as tile
from concourse import bacc, mybir
from concourse.bass_utils import run_bass_kernel_spmd
from concourse import dve_ops as _dops
from concourse.dve_spec import Spec, Src0, Src1, scan, AluOp, lower
from concourse.dve_spec import _has_src1 as _hs1
from concourse.dve_uop import DveOpSpec

F32 = mybir.dt.float32
F16 = mybir.dt.float16
F8 = mybir.dt.float8e4

N_CORES = 8
BATCH, H, W = 128, 512, 512
IMGS_PER_CORE = BATCH // N_CORES      # 16
BLK = 128
RB = 118                              # real rows per block
NSEG = 5                              # blocks per image
RLAST = H - 4 * RB                    # 40 rows in the last block
K = 11
PAD = K // 2                          # 5
ZH = K                                # zero head width
WT = ZH + PAD + W + PAD               # 533 segment width
X0 = ZH + PAD                         # x offset within segment (16)
FLAT = NSEG * WT                      # 2665
SCLEN = FLAT - ZH                     # 2654 scan steps
ROWS = IMGS_PER_CORE * BLK            # 2048 partition-rows per core
OROW = NSEG * RB                      # 590 output rows per image (incl pad)
CN = ("bmh", "bm4", "idn")


def _register_wscan():
    name = "ADAPT_WSCAN"
    if name in _dops._SUB_OPCODE_FOR_NAME:
        return next(o for o in _dops.OPS if o.name == name)
    spec = Spec(
        body=scan(AluOp.ADD, Src0 - Src1),
        reference=lambda in0, in1, s0, s1, imm2: np.cumsum(
            in0.astype(np.float32) - in1.astype(np.float32), axis=-1),
    )
    row = _dops._CUSTOM_DVE_ROW_BASE + len(_dops.OPS)
    _dops._SUB_OPCODE_FOR_NAME[name] = row
    shas = {}
    for ver in ("v3", "v4"):
        tmp = DveOpSpec(name=name, opcode=row, uops=lower(spec, ver=ver),
                        rd1_en=_hs1(spec))
        shas[ver] = tmp.sha(ver)
    op = _dops.DveOp(name, spec, subdim=False, uops_sha=shas)
    _dops.OPS.append(op)
    _dops.CUSTOM_DVE_SPECS[name] = spec
    return op


def _band_matrices(dtype=np.float16):
    # uniform fused band+halo matrix: moving partition k -> output row r
    bmh = np.zeros((BLK, BLK), dtype=dtype)
    for r in range(RB):
        for k in range(RB):
            if abs(r - k) <= PAD:
                bmh[k, r] = 1.0
        for j in range(PAD):
            if r >= RB - PAD + j:     # next rows (partitions 118+j)
                bmh[RB + j, r] = 1.0
            if r <= j:                # prev rows (partitions 123+j)
                bmh[RB + PAD + j, r] = 1.0
    # block4 variant: only RLAST real rows, so the next-row (replicate 511)
    # taps activate at r >= RLAST-PAD+j instead of RB-PAD+j
    bm4 = np.zeros((BLK, BLK), dtype=dtype)
    for r in range(RLAST):
        for k in range(RB):
            if abs(r - k) <= PAD:
                bm4[k, r] = 1.0
        for j in range(PAD):
            if r >= RLAST - PAD + j:
                bm4[RB + j, r] = 1.0
            if r <= j:
                bm4[RB + PAD + j, r] = 1.0
    idn = (-121.0 * np.eye(BLK)).astype(dtype)
    return {"bmh": bmh, "bm4": bm4, "idn": idn}


def _build():
    wop = _register_wscan()
    nc = bacc.Bacc(None, target_bir_lowering=False, debug=False)
    x_d = nc.declare_dram_parameter("x", [ROWS, FLAT], F16, isOutput=False)
    c_d = nc.declare_dram_parameter("consts", [BLK, len(CN) * BLK], F16,
                                    isOutput=False)
    out_d = nc.declare_dram_parameter("out", [IMGS_PER_CORE * OROW, W], F8,
                                      isOutput=True)

    with tile.TileContext(nc) as tc:
        with (
            tc.tile_pool(name="cpool", bufs=1) as cpool,
            tc.tile_pool(name="xin", bufs=4) as x_pool,
            tc.tile_pool(name="scr", bufs=4) as s_pool,
            tc.tile_pool(name="outp", bufs=3) as o_pool,
            tc.tile_pool(name="psA", bufs=2, space=bass.MemorySpace.PSUM) as pa_pool,
            tc.tile_pool(name="psB", bufs=1, space=bass.MemorySpace.PSUM) as pb_pool,
        ):
            cbig = cpool.tile([BLK, len(CN) * BLK], F16, tag="consts")
            nc.scalar.dma_start(cbig[:], c_d[:])
            ct = {nm: cbig[:, j * BLK:(j + 1) * BLK] for j, nm in enumerate(CN)}
            bias_t = cpool.tile([BLK, 1], F32, tag="bias")
            nc.vector.memset(bias_t[:], -242.0 / 4.0)

            imgs = {}
            EDGE = (0, IMGS_PER_CORE - 1)

            def segof(pos):
                return pos * WT + (K - 1)

            def front_img(i):
                ximg = x_pool.tile([BLK, NSEG, WT], F16, tag="ximg")
                r0 = i * BLK
                # block4: zero dead partitions 40-117 (whole-plane memset,
                # the live-row DMAs below overwrite their slices)
                nc.gpsimd.memset(ximg[:, NSEG - 1, :], 0.0)
                if i in EDGE:
                    for pos in range(NSEG - 1):
                        nc.sync.dma_start(
                            ximg[:, pos, :],
                            x_d[r0:r0 + BLK, pos * WT:(pos + 1) * WT])
                else:
                    nc.sync.dma_start(
                        ximg[:, 0:NSEG - 1, :],
                        x_d[r0:r0 + BLK, 0:(NSEG - 1) * WT].rearrange(
                            "q (p c) -> q p c", p=NSEG - 1))
                nc.sync.dma_start(
                    ximg[0:RLAST, NSEG - 1, :],
                    x_d[r0:r0 + RLAST, (NSEG - 1) * WT:FLAT])
                nc.gpsimd.dma_start(
                    ximg[RB:BLK, NSEG - 1, :],
                    x_d[r0 + RB:r0 + BLK, (NSEG - 1) * WT:FLAT])
                s = s_pool.tile([BLK, SCLEN], F16, tag="scr")
                flat = ximg[:].rearrange("q p c -> q (p c)")
                if i in EDGE:
                    for pos in range(NSEG):
                        o0 = pos * WT
                        nc.vector._custom_dve(
                            wop, out=s[:, o0:o0 + WT - ZH],
                            in0=flat[:, o0 + ZH:o0 + WT],
                            in1=flat[:, o0:o0 + WT - ZH])
                else:
                    nc.vector._custom_dve(
                        wop, out=s[:], in0=flat[:, ZH:FLAT],
                        in1=flat[:, 0:SCLEN])
                imgs[i] = (ximg, s)

            def back_img(i):
                ximg, s = imgs.pop(i)
                psA = pa_pool.tile([BLK, 3, W], F32, tag="psA", name=f"psA_{i}")
                psB = pb_pool.tile([BLK, 2, W], F32, tag="psB", name=f"psB_{i}")
                flat = ximg[:].rearrange("q p c -> q (p c)")
                sseg = [s[:, segof(p):segof(p) + W] for p in range(NSEG)]
                xseg = [flat[:, p * WT + X0:p * WT + X0 + W] for p in range(NSEG)]

                def psof(b):
                    return psA[0:RB, b, :] if b < 3 else psB[0:RB, b - 3, :]

                def mm_idn(b):
                    nc.tensor.matmul(psof(b), ct["idn"][0:RB, 0:RB],
                                     ximg[0:RB, b, X0:X0 + W],
                                     start=True, stop=False)

                def mm_bmh(b):
                    w = ct["bm4"] if b == NSEG - 1 else ct["bmh"]
                    nc.tensor.matmul(psof(b), w[:, 0:RB], sseg[b],
                                     start=False, stop=True)

                # B tile (single buffer) first so its ACT frees it early
                for b in (3, 4):
                    mm_idn(b)
                for b in (3, 4):
                    mm_bmh(b)
                for b in (0, 1, 2):
                    mm_idn(b)
                for b in (0, 1, 2):
                    mm_bmh(b)

                oimg = o_pool.tile([BLK, NSEG, W], F8, tag="oimg")
                orow = out_d[i * OROW:(i + 1) * OROW, :].rearrange(
                    "(p q) c -> q p c", p=NSEG)
                nc.scalar.activation(
                    oimg[0:RB, 3:5, :], psB[0:RB, :, :],
                    mybir.ActivationFunctionType.Sign,
                    bias=bias_t[0:RB], scale=1.0)
                nc.sync.dma_start(orow[0:RB, 3:5, :], oimg[0:RB, 3:5, :])
                nc.scalar.activation(
                    oimg[0:RB, 0:3, :], psA[0:RB, :, :],
                    mybir.ActivationFunctionType.Sign,
                    bias=bias_t[0:RB], scale=1.0)
                nc.sync.dma_start(orow[0:RB, 0:3, :], oimg[0:RB, 0:3, :])

            front_img(0)
            front_img(1)
            front_img(2)
            for i in range(IMGS_PER_CORE):
                back_img(i)
                if i + 3 < IMGS_PER_CORE:
                    front_img(i + 3)
    nc.compile()
    return nc


_NC_CACHE = None


def _make_in_maps(x: np.ndarray) -> list:
    x = np.asarray(x, dtype=np.float32)
    y = (x.reshape(BATCH, H, W).astype(np.float16) / np.float16(4.0))
    # partition index -> image row per block (with vertical replicate)
    idx = np.zeros((NSEG, BLK), dtype=np.int64)
    live = np.zeros((NSEG, BLK), dtype=bool)
    for b in range(NSEG):
        nr = RB if b < NSEG - 1 else RLAST
        base = b * RB
        idx[b, 0:nr] = base + np.arange(nr)
        live[b, 0:nr] = True
        idx[b, RB:RB + PAD] = np.clip(base + nr + np.arange(PAD), 0, H - 1) \
            if b == NSEG - 1 else base + RB + np.arange(PAD)
        live[b, RB:RB + PAD] = True
        idx[b, RB + PAD:BLK] = np.clip(base - PAD + np.arange(PAD), 0, H - 1)
        live[b, RB + PAD:BLK] = True
    planes = y[:, idx, :]                       # [B, NSEG, BLK, W]
    planes[:, ~live, :] = np.float16(0.0)
    plane = np.zeros((BATCH, NSEG, BLK, WT), dtype=np.float16)
    plane[..., X0:X0 + W] = planes
    plane[..., ZH:X0] = planes[..., 0:1]
    plane[..., X0 + W:WT] = planes[..., W - 1:W]
    plane = plane.transpose(0, 2, 1, 3)         # [B, BLK, NSEG, WT]
    cm = _band_matrices()
    cbig = np.concatenate([cm[nm] for nm in CN], axis=1)
    in_maps = []
    for c in range(N_CORES):
        shard = plane[c * IMGS_PER_CORE:(c + 1) * IMGS_PER_CORE].reshape(
            ROWS, FLAT)
        in_maps.append({"x": np.ascontiguousarray(shard),
                        "consts": np.ascontiguousarray(cbig)})
    return in_maps


def kernel(x: np.ndarray) -> np.ndarray:
    global _NC_CACHE
    if _NC_CACHE is None:
        _NC_CACHE = _build()
    nc = _NC_CACHE
    in_maps = _make_in_maps(x)
    res = run_bass_kernel_spmd(nc, in_maps, core_ids=list(range(N_CORES)))
    out = np.empty((BATCH, H, W), dtype=np.float32)
    for c in range(N_CORES):
        sgn = np.asarray(res.results[c]["out"]).view(np.uint8)
        o = (sgn < 0x80).reshape(IMGS_PER_CORE, NSEG, RB, W)
        for b in range(NSEG):
            nr = RB if b < NSEG - 1 else RLAST
            out[c * IMGS_PER_CORE:(c + 1) * IMGS_PER_CORE,
                b * RB:b * RB + nr] = o[:, b, 0:nr].astype(np.float32) * 255.0
    return out.reshape(BATCH, H, W, 1)


# revision 18
# speedup vs baseline: 1.1213x; 1.1213x over previous
"""Adaptive thresholding (11x11 box mean, BORDER_REPLICATE, THRESH_BINARY_INV)
on 8 TRN2 NeuronCores, data-parallel over the batch dim (16 images/core).

v7 design — fused-halo 118-row blocks:
  - Each image is split into 5 row-blocks of [118,118,118,118,40] rows. For
    block b the host packs a [128, 533] fp16 segment: partitions 0-117 the
    block rows, 118-122 the next 5 rows, 123-127 the previous 5 rows
    (BORDER_REPLICATE at the image edges), all scaled y = fp16(x)/4 with an
    11-col zero head and 5-col horizontal replicate margins. Vertical halos
    and edge handling thus live in the DATA, so ONE uniform band matrix
    bmh [128,118] computes the complete vertical sum per block: 2 matmuls
    per block (idn + bmh), 10 per image, 2 LDWEIGHTS groups, and no
    cross-segment PE dependencies.
  - Custom DVE op ADAPT_WSCAN: out = inclusive_scan_add(Src0 - Src1) at
    1 elem/cycle; one scan per image over the flat [128, 5*533] view; the
    duplicated halo partitions are scanned for free (partition-parallel).
  - Block4's dead partitions 40-117 are zeroed on-chip (gpsimd memset) and
    skipped by the DMA (the x plane for block4 is fetched as two partition
    slices).
  - PSUM: tile A [128,3,512] (blocks 0-2, 2 bufs) + tile B [128,2,512]
    (blocks 3-4, 1 buf) = exactly 8 banks. Per image: Sign(PSUM - 60.5)
    as two merged activations (B first so its single buffer recycles
    early) -> fp8e4m3, out at 1 byte/px.
Host: out = (sign >= 0) * 255  (inclusive compare matches x <= mean-2).
"""
import sys
sys.path.insert(0, '/opt/trn_rl_repo')
import numpy as np
import concourse.bass as bass
import concourse.tile as tile
from concourse import bacc, mybir
from concourse.bass_utils import run_bass_kernel_spmd
from concourse import dve_ops as _dops
from concourse.dve_spec import Spec, Src0, Src1, scan, AluOp, lower
from concourse.dve_spec import _has_src1 as _hs1
from concourse.dve_uop import DveOpSpec

F32 = mybir.dt.float32
F16 = mybir.dt.float16
F8 = mybir.dt.float8e4

N_CORES = 8
BATCH, H, W = 128, 512, 512
IMGS_PER_CORE = BATCH // N_CORES      # 16
BLK = 128
RB = 118                              # real rows per block
NSEG = 5                              # blocks per image
RLAST = H - 4 * RB                    # 40 rows in the last block
K = 11
PAD = K // 2                          # 5
ZH = K                                # zero head width
WT = ZH + PAD + W + PAD               # 533 segment width
X0 = ZH + PAD                         # x offset within segment (16)
FLAT = NSEG * WT                      # 2665
SCLEN = FLAT - ZH                     # 2654 scan steps
ROWS = IMGS_PER_CORE * BLK            # 2048 partition-rows per core
OROW = NSEG * RB                      # 590 output rows per image (incl pad)
CN = ("bmh", "bm4", "idn")


def _register_wscan():
    name = "ADAPT_WSCAN"
    if name in _dops._SUB_OPCODE_FOR_NAME:
        return next(o for o in _dops.OPS if o.name == name)
    spec = Spec(
        body=scan(AluOp.ADD, Src0 - Src1),
        reference=lambda in0, in1, s0, s1, imm2: np.cumsum(
            in0.astype(np.float32) - in1.astype(np.float32), axis=-1),
    )
    row = _dops._CUSTOM_DVE_ROW_BASE + len(_dops.OPS)
    _dops._SUB_OPCODE_FOR_NAME[name] = row
    shas = {}
    for ver in ("v3", "v4"):
        tmp = DveOpSpec(name=name, opcode=row, uops=lower(spec, ver=ver),
                        rd1_en=_hs1(spec))
        shas[ver] = tmp.sha(ver)
    op = _dops.DveOp(name, spec, subdim=False, uops_sha=shas)
    _dops.OPS.append(op)
    _dops.CUSTOM_DVE_SPECS[name] = spec
    return op


def _band_matrices(dtype=np.float16):
    # uniform fused band+halo matrix: moving partition k -> output row r
    bmh = np.zeros((BLK, BLK), dtype=dtype)
    for r in range(RB):
        for k in range(RB):
            if abs(r - k) <= PAD:
                bmh[k, r] = 1.0
        for j in range(PAD):
            if r >= RB - PAD + j:     # next rows (partitions 118+j)
                bmh[RB + j, r] = 1.0
            if r <= j:                # prev rows (partitions 123+j)
                bmh[RB + PAD + j, r] = 1.0
    # block4 variant: only RLAST real rows, so the next-row (replicate 511)
    # taps activate at r >= RLAST-PAD+j instead of RB-PAD+j
    bm4 = np.zeros((BLK, BLK), dtype=dtype)
    for r in range(RLAST):
        for k in range(RB):
            if abs(r - k) <= PAD:
                bm4[k, r] = 1.0
        for j in range(PAD):
            if r >= RLAST - PAD + j:
                bm4[RB + j, r] = 1.0
            if r <= j:
                bm4[RB + PAD + j, r] = 1.0
    idn = (-121.0 * np.eye(BLK)).astype(dtype)
    return {"bmh": bmh, "bm4": bm4, "idn": idn}


def _build():
    wop = _register_wscan()
    nc = bacc.Bacc(None, target_bir_lowering=False, debug=False)
    x_d = nc.declare_dram_parameter("x", [ROWS, FLAT], F16, isOutput=False)
    c_d = nc.declare_dram_parameter("consts", [BLK, len(CN) * BLK], F16,
                                    isOutput=False)
    out_d = nc.declare_dram_parameter("out", [ROWS, NSEG * W], F8,
                                      isOutput=True)

    with tile.TileContext(nc) as tc:
        with (
            tc.tile_pool(name="cpool", bufs=1) as cpool,
            tc.tile_pool(name="xin", bufs=4) as x_pool,
            tc.tile_pool(name="scr", bufs=4) as s_pool,
            tc.tile_pool(name="outp", bufs=3) as o_pool,
            tc.tile_pool(name="psA", bufs=2, space=bass.MemorySpace.PSUM) as pa_pool,
            tc.tile_pool(name="psB", bufs=1, space=bass.MemorySpace.PSUM) as pb_pool,
        ):
            cbig = cpool.tile([BLK, len(CN) * BLK], F16, tag="consts")
            nc.scalar.dma_start(cbig[:], c_d[:])
            ct = {nm: cbig[:, j * BLK:(j + 1) * BLK] for j, nm in enumerate(CN)}
            bias_t = cpool.tile([BLK, 1], F32, tag="bias")
            nc.vector.memset(bias_t[:], -242.0 / 4.0)

            imgs = {}
            EDGE = (0, IMGS_PER_CORE - 1)

            def segof(pos):
                return pos * WT + (K - 1)

            def front_img(i):
                ximg = x_pool.tile([BLK, NSEG, WT], F16, tag="ximg")
                r0 = i * BLK
                # block4: zero dead partitions 40-117 (whole-plane memset,
                # the live-row DMAs below overwrite their slices)
                nc.gpsimd.memset(ximg[:, NSEG - 1, :], 0.0)
                if i in EDGE:
                    for pos in range(NSEG - 1):
                        nc.sync.dma_start(
                            ximg[:, pos, :],
                            x_d[r0:r0 + BLK, pos * WT:(pos + 1) * WT])
                else:
                    nc.sync.dma_start(
                        ximg[:, 0:NSEG - 1, :],
                        x_d[r0:r0 + BLK, 0:(NSEG - 1) * WT].rearrange(
                            "q (p c) -> q p c", p=NSEG - 1))
                nc.gpsimd.dma_start(
                    ximg[0:RLAST, NSEG - 1, :],
                    x_d[r0:r0 + RLAST, (NSEG - 1) * WT:FLAT])
                nc.gpsimd.dma_start(
                    ximg[RB:BLK, NSEG - 1, :],
                    x_d[r0 + RB:r0 + BLK, (NSEG - 1) * WT:FLAT])
                s = s_pool.tile([BLK, SCLEN], F16, tag="scr")
                flat = ximg[:].rearrange("q p c -> q (p c)")
                if i in EDGE:
                    for pos in range(NSEG):
                        o0 = pos * WT
                        nc.vector._custom_dve(
                            wop, out=s[:, o0:o0 + WT - ZH],
                            in0=flat[:, o0 + ZH:o0 + WT],
                            in1=flat[:, o0:o0 + WT - ZH])
                else:
                    nc.vector._custom_dve(
                        wop, out=s[:], in0=flat[:, ZH:FLAT],
                        in1=flat[:, 0:SCLEN])
                imgs[i] = (ximg, s)

            def back_img(i):
                ximg, s = imgs.pop(i)
                psA = pa_pool.tile([BLK, 3, W], F32, tag="psA", name=f"psA_{i}")
                psB = pb_pool.tile([BLK, 2, W], F32, tag="psB", name=f"psB_{i}")
                flat = ximg[:].rearrange("q p c -> q (p c)")
                sseg = [s[:, segof(p):segof(p) + W] for p in range(NSEG)]
                xseg = [flat[:, p * WT + X0:p * WT + X0 + W] for p in range(NSEG)]

                def psof(b):
                    return psA[0:RB, b, :] if b < 3 else psB[0:RB, b - 3, :]

                def mm_idn(b):
                    nc.tensor.matmul(psof(b), ct["idn"][0:RB, 0:RB],
                                     ximg[0:RB, b, X0:X0 + W],
                                     start=True, stop=False)

                def mm_bmh(b):
                    w = ct["bm4"] if b == NSEG - 1 else ct["bmh"]
                    nc.tensor.matmul(psof(b), w[:, 0:RB], sseg[b],
                                     start=False, stop=True)

                # B tile (single buffer) first so its ACT frees it early
                for b in (3, 4):
                    mm_idn(b)
                for b in (3, 4):
                    mm_bmh(b)
                for b in (0, 1, 2):
                    mm_idn(b)
                for b in (0, 1, 2):
                    mm_bmh(b)

                oimg = o_pool.tile([BLK, NSEG, W], F8, tag="oimg")
                orow = out_d[i * BLK:(i + 1) * BLK, :].rearrange(
                    "q (p c) -> q p c", p=NSEG)
                nc.scalar.activation(
                    oimg[0:RB, 3:5, :], psB[0:RB, :, :],
                    mybir.ActivationFunctionType.Sign,
                    bias=bias_t[0:RB], scale=1.0)
                nc.sync.dma_start(orow[0:RB, 3:5, :], oimg[0:RB, 3:5, :])
                nc.scalar.activation(
                    oimg[0:RB, 0:3, :], psA[0:RB, :, :],
                    mybir.ActivationFunctionType.Sign,
                    bias=bias_t[0:RB], scale=1.0)
                nc.sync.dma_start(orow[0:RB, 0:3, :], oimg[0:RB, 0:3, :])

            front_img(0)
            front_img(1)
            front_img(2)
            for i in range(IMGS_PER_CORE):
                back_img(i)
                if i + 3 < IMGS_PER_CORE:
                    front_img(i + 3)
    nc.compile()
    return nc


_NC_CACHE = None


def _make_in_maps(x: np.ndarray) -> list:
    x = np.asarray(x, dtype=np.float32)
    y = (x.reshape(BATCH, H, W).astype(np.float16) / np.float16(4.0))
    # partition index -> image row per block (with vertical replicate)
    idx = np.zeros((NSEG, BLK), dtype=np.int64)
    live = np.zeros((NSEG, BLK), dtype=bool)
    for b in range(NSEG):
        nr = RB if b < NSEG - 1 else RLAST
        base = b * RB
        idx[b, 0:nr] = base + np.arange(nr)
        live[b, 0:nr] = True
        idx[b, RB:RB + PAD] = np.clip(base + nr + np.arange(PAD), 0, H - 1) \
            if b == NSEG - 1 else base + RB + np.arange(PAD)
        live[b, RB:RB + PAD] = True
        idx[b, RB + PAD:BLK] = np.clip(base - PAD + np.arange(PAD), 0, H - 1)
        live[b, RB + PAD:BLK] = True
    planes = y[:, idx, :]                       # [B, NSEG, BLK, W]
    planes[:, ~live, :] = np.float16(0.0)
    plane = np.zeros((BATCH, NSEG, BLK, WT), dtype=np.float16)
    plane[..., X0:X0 + W] = planes
    plane[..., ZH:X0] = planes[..., 0:1]
    plane[..., X0 + W:WT] = planes[..., W - 1:W]
    plane = plane.transpose(0, 2, 1, 3)         # [B, BLK, NSEG, WT]
    cm = _band_matrices()
    cbig = np.concatenate([cm[nm] for nm in CN], axis=1)
    in_maps = []
    for c in range(N_CORES):
        shard = plane[c * IMGS_PER_CORE:(c + 1) * IMGS_PER_CORE].reshape(
            ROWS, FLAT)
        in_maps.append({"x": np.ascontiguousarray(shard),
                        "consts": np.ascontiguousarray(cbig)})
    return in_maps


def kernel(x: np.ndarray) -> np.ndarray:
    global _NC_CACHE
    if _NC_CACHE is None:
        _NC_CACHE = _build()
    nc = _NC_CACHE
    in_maps = _make_in_maps(x)
    res = run_bass_kernel_spmd(nc, in_maps, core_ids=list(range(N_CORES)))
    out = np.empty((BATCH, H, W), dtype=np.float32)
    for c in range(N_CORES):
        sgn = np.asarray(res.results[c]["out"]).view(np.uint8)
        o = (sgn < 0x80).reshape(IMGS_PER_CORE, BLK, NSEG, W)
        for b in range(NSEG):
            nr = RB if b < NSEG - 1 else RLAST
            out[c * IMGS_PER_CORE:(c + 1) * IMGS_PER_CORE,
                b * RB:b * RB + nr] = \
                o[:, 0:nr, b].astype(np.float32) * 255.0
    return out.reshape(BATCH, H, W, 1)


# revision 19
# speedup vs baseline: 1.1506x; 1.0262x over previous
"""Adaptive thresholding (11x11 box mean, BORDER_REPLICATE, THRESH_BINARY_INV)
on 8 TRN2 NeuronCores, data-parallel over the batch dim (16 images/core).

v7 design — fused-halo 118-row blocks:
  - Each image is split into 5 row-blocks of [118,118,118,118,40] rows. For
    block b the host packs a [128, 533] fp16 segment: partitions 0-117 the
    block rows, 118-122 the next 5 rows, 123-127 the previous 5 rows
    (BORDER_REPLICATE at the image edges), all scaled y = fp16(x)/4 with an
    11-col zero head and 5-col horizontal replicate margins. Vertical halos
    and edge handling thus live in the DATA, so ONE uniform band matrix
    bmh [128,118] computes the complete vertical sum per block: 2 matmuls
    per block (idn + bmh), 10 per image, 2 LDWEIGHTS groups, and no
    cross-segment PE dependencies.
  - Custom DVE op ADAPT_WSCAN: out = inclusive_scan_add(Src0 - Src1) at
    1 elem/cycle; one scan per image over the flat [128, 5*533] view; the
    duplicated halo partitions are scanned for free (partition-parallel).
  - Block4's dead partitions 40-117 are zeroed on-chip (gpsimd memset) and
    skipped by the DMA (the x plane for block4 is fetched as two partition
    slices).
  - PSUM: tile A [128,3,512] (blocks 0-2, 2 bufs) + tile B [128,2,512]
    (blocks 3-4, 1 buf) = exactly 8 banks. Per image: Sign(PSUM - 60.5)
    as two merged activations (B first so its single buffer recycles
    early) -> fp8e4m3, out at 1 byte/px.
Host: out = (sign >= 0) * 255  (inclusive compare matches x <= mean-2).
"""
import sys
sys.path.insert(0, '/opt/trn_rl_repo')
import numpy as np
import concourse.bass as bass
import concourse.tile as tile
from concourse import bacc, mybir
from concourse.bass_utils import run_bass_kernel_spmd
from concourse import dve_ops as _dops
from concourse.dve_spec import Spec, Src0, Src1, scan, AluOp, lower
from concourse.dve_spec import _has_src1 as _hs1
from concourse.dve_uop import DveOpSpec

F32 = mybir.dt.float32
F16 = mybir.dt.float16
F8 = mybir.dt.float8e4

N_CORES = 8
BATCH, H, W = 128, 512, 512
IMGS_PER_CORE = BATCH // N_CORES      # 16
BLK = 128
RB = 118                              # real rows per block
NSEG = 5                              # blocks per image
RLAST = H - 4 * RB                    # 40 rows in the last block
K = 11
PAD = K // 2                          # 5
ZH = K                                # zero head width
WT = ZH + PAD + W + PAD               # 533 segment width
X0 = ZH + PAD                         # x offset within segment (16)
FLAT = NSEG * WT                      # 2665
SCLEN = FLAT - ZH                     # 2654 scan steps
ROWS = IMGS_PER_CORE * BLK            # 2048 partition-rows per core
OROW = NSEG * RB                      # 590 output rows per image (incl pad)
CN = ("bmh", "bm4", "idn")


def _register_wscan():
    name = "ADAPT_WSCAN"
    if name in _dops._SUB_OPCODE_FOR_NAME:
        return next(o for o in _dops.OPS if o.name == name)
    spec = Spec(
        body=scan(AluOp.ADD, Src0 - Src1),
        reference=lambda in0, in1, s0, s1, imm2: np.cumsum(
            in0.astype(np.float32) - in1.astype(np.float32), axis=-1),
    )
    row = _dops._CUSTOM_DVE_ROW_BASE + len(_dops.OPS)
    _dops._SUB_OPCODE_FOR_NAME[name] = row
    shas = {}
    for ver in ("v3", "v4"):
        tmp = DveOpSpec(name=name, opcode=row, uops=lower(spec, ver=ver),
                        rd1_en=_hs1(spec))
        shas[ver] = tmp.sha(ver)
    op = _dops.DveOp(name, spec, subdim=False, uops_sha=shas)
    _dops.OPS.append(op)
    _dops.CUSTOM_DVE_SPECS[name] = spec
    return op


def _band_matrices(dtype=np.float16):
    # uniform fused band+halo matrix: moving partition k -> output row r
    bmh = np.zeros((BLK, BLK), dtype=dtype)
    for r in range(RB):
        for k in range(RB):
            if abs(r - k) <= PAD:
                bmh[k, r] = 1.0
        for j in range(PAD):
            if r >= RB - PAD + j:     # next rows (partitions 118+j)
                bmh[RB + j, r] = 1.0
            if r <= j:                # prev rows (partitions 123+j)
                bmh[RB + PAD + j, r] = 1.0
    # block4 variant: only RLAST real rows, so the next-row (replicate 511)
    # taps activate at r >= RLAST-PAD+j instead of RB-PAD+j
    bm4 = np.zeros((BLK, BLK), dtype=dtype)
    for r in range(RLAST):
        for k in range(RLAST):
            if abs(r - k) <= PAD:
                bm4[k, r] = 1.0
        for j in range(PAD):
            if r >= RLAST - PAD + j:
                bm4[RLAST + j, r] = 1.0
            if r <= j:
                bm4[RLAST + PAD + j, r] = 1.0
    idn = (-121.0 * np.eye(BLK)).astype(dtype)
    return {"bmh": bmh, "bm4": bm4, "idn": idn}


def _build():
    wop = _register_wscan()
    nc = bacc.Bacc(None, target_bir_lowering=False, debug=False)
    x_d = nc.declare_dram_parameter("x", [ROWS, FLAT], F16, isOutput=False)
    c_d = nc.declare_dram_parameter("consts", [BLK, len(CN) * BLK], F16,
                                    isOutput=False)
    out_d = nc.declare_dram_parameter("out", [ROWS, NSEG * W], F8,
                                      isOutput=True)

    with tile.TileContext(nc) as tc:
        with (
            tc.tile_pool(name="cpool", bufs=1) as cpool,
            tc.tile_pool(name="xin", bufs=4) as x_pool,
            tc.tile_pool(name="scr", bufs=4) as s_pool,
            tc.tile_pool(name="outp", bufs=3) as o_pool,
            tc.tile_pool(name="psA", bufs=2, space=bass.MemorySpace.PSUM) as pa_pool,
            tc.tile_pool(name="psB", bufs=1, space=bass.MemorySpace.PSUM) as pb_pool,
        ):
            cbig = cpool.tile([BLK, len(CN) * BLK], F16, tag="consts")
            nc.scalar.dma_start(cbig[:], c_d[:])
            ct = {nm: cbig[:, j * BLK:(j + 1) * BLK] for j, nm in enumerate(CN)}
            bias_t = cpool.tile([BLK, 1], F32, tag="bias")
            nc.vector.memset(bias_t[:], -242.0 / 4.0)

            imgs = {}
            EDGE = (0, IMGS_PER_CORE - 1)

            def segof(pos):
                return pos * WT + (K - 1)

            def front_img(i):
                ximg = x_pool.tile([BLK, NSEG, WT], F16, tag="ximg")
                r0 = i * BLK
                # block4: zero dead partitions 40-117 (whole-plane memset,
                # the live-row DMAs below overwrite their slices)
                nc.gpsimd.memset(ximg[:, NSEG - 1, :], 0.0)
                if i in EDGE:
                    for pos in range(NSEG - 1):
                        nc.sync.dma_start(
                            ximg[:, pos, :],
                            x_d[r0:r0 + BLK, pos * WT:(pos + 1) * WT])
                else:
                    nc.sync.dma_start(
                        ximg[:, 0:NSEG - 1, :],
                        x_d[r0:r0 + BLK, 0:(NSEG - 1) * WT].rearrange(
                            "q (p c) -> q p c", p=NSEG - 1))
                nc.gpsimd.dma_start(
                    ximg[0:RLAST + 2 * PAD, NSEG - 1, :],
                    x_d[r0:r0 + RLAST + 2 * PAD, (NSEG - 1) * WT:FLAT])
                s = s_pool.tile([BLK, SCLEN], F16, tag="scr")
                flat = ximg[:].rearrange("q p c -> q (p c)")
                if i in EDGE:
                    for pos in range(NSEG):
                        o0 = pos * WT
                        nc.vector._custom_dve(
                            wop, out=s[:, o0:o0 + WT - ZH],
                            in0=flat[:, o0 + ZH:o0 + WT],
                            in1=flat[:, o0:o0 + WT - ZH])
                else:
                    nc.vector._custom_dve(
                        wop, out=s[:], in0=flat[:, ZH:FLAT],
                        in1=flat[:, 0:SCLEN])
                imgs[i] = (ximg, s)

            def back_img(i):
                ximg, s = imgs.pop(i)
                psA = pa_pool.tile([BLK, 3, W], F32, tag="psA", name=f"psA_{i}")
                psB = pb_pool.tile([BLK, 2, W], F32, tag="psB", name=f"psB_{i}")
                flat = ximg[:].rearrange("q p c -> q (p c)")
                sseg = [s[:, segof(p):segof(p) + W] for p in range(NSEG)]
                xseg = [flat[:, p * WT + X0:p * WT + X0 + W] for p in range(NSEG)]

                def psof(b):
                    return psA[0:RB, b, :] if b < 3 else psB[0:RB, b - 3, :]

                def mm_idn(b):
                    nc.tensor.matmul(psof(b), ct["idn"][0:RB, 0:RB],
                                     ximg[0:RB, b, X0:X0 + W],
                                     start=True, stop=False)

                def mm_bmh(b):
                    w = ct["bm4"] if b == NSEG - 1 else ct["bmh"]
                    nc.tensor.matmul(psof(b), w[:, 0:RB], sseg[b],
                                     start=False, stop=True)

                # B tile (single buffer) first so its ACT frees it early
                for b in (3, 4):
                    mm_idn(b)
                for b in (3, 4):
                    mm_bmh(b)
                for b in (0, 1, 2):
                    mm_idn(b)
                for b in (0, 1, 2):
                    mm_bmh(b)

                oimg = o_pool.tile([BLK, NSEG, W], F8, tag="oimg")
                orow = out_d[i * BLK:(i + 1) * BLK, :].rearrange(
                    "q (p c) -> q p c", p=NSEG)
                nc.scalar.activation(
                    oimg[0:RB, 3:5, :], psB[0:RB, :, :],
                    mybir.ActivationFunctionType.Sign,
                    bias=bias_t[0:RB], scale=1.0)
                nc.scalar.activation(
                    oimg[0:RB, 0:3, :], psA[0:RB, :, :],
                    mybir.ActivationFunctionType.Sign,
                    bias=bias_t[0:RB], scale=1.0)
                nc.sync.dma_start(orow[0:RB, :, :], oimg[0:RB, :, :])

            front_img(0)
            front_img(1)
            front_img(2)
            for i in range(IMGS_PER_CORE):
                back_img(i)
                if i + 3 < IMGS_PER_CORE:
                    front_img(i + 3)
    nc.compile()
    return nc


_NC_CACHE = None


def _make_in_maps(x: np.ndarray) -> list:
    x = np.asarray(x, dtype=np.float32)
    y = (x.reshape(BATCH, H, W).astype(np.float16) / np.float16(4.0))
    # partition index -> image row per block (with vertical replicate)
    idx = np.zeros((NSEG, BLK), dtype=np.int64)
    live = np.zeros((NSEG, BLK), dtype=bool)
    for b in range(NSEG):
        nr = RB if b < NSEG - 1 else RLAST
        base = b * RB
        idx[b, 0:nr] = base + np.arange(nr)
        live[b, 0:nr] = True
        idx[b, nr:nr + PAD] = np.clip(base + nr + np.arange(PAD), 0, H - 1)
        live[b, nr:nr + PAD] = True
        idx[b, nr + PAD:nr + 2 * PAD] = np.clip(
            base - PAD + np.arange(PAD), 0, H - 1)
        live[b, nr + PAD:nr + 2 * PAD] = True
    planes = y[:, idx, :]                       # [B, NSEG, BLK, W]
    planes[:, ~live, :] = np.float16(0.0)
    plane = np.zeros((BATCH, NSEG, BLK, WT), dtype=np.float16)
    plane[..., X0:X0 + W] = planes
    plane[..., ZH:X0] = planes[..., 0:1]
    plane[..., X0 + W:WT] = planes[..., W - 1:W]
    plane = plane.transpose(0, 2, 1, 3)         # [B, BLK, NSEG, WT]
    cm = _band_matrices()
    cbig = np.concatenate([cm[nm] for nm in CN], axis=1)
    in_maps = []
    for c in range(N_CORES):
        shard = plane[c * IMGS_PER_CORE:(c + 1) * IMGS_PER_CORE].reshape(
            ROWS, FLAT)
        in_maps.append({"x": np.ascontiguousarray(shard),
                        "consts": np.ascontiguousarray(cbig)})
    return in_maps


def kernel(x: np.ndarray) -> np.ndarray:
    global _NC_CACHE
    if _NC_CACHE is None:
        _NC_CACHE = _build()
    nc = _NC_CACHE
    in_maps = _make_in_maps(x)
    res = run_bass_kernel_spmd(nc, in_maps, core_ids=list(range(N_CORES)))
    out = np.empty((BATCH, H, W), dtype=np.float32)
    for c in range(N_CORES):
        sgn = np.asarray(res.results[c]["out"]).view(np.uint8)
        o = (sgn < 0x80).reshape(IMGS_PER_CORE, BLK, NSEG, W)
        for b in range(NSEG):
            nr = RB if b < NSEG - 1 else RLAST
            out[c * IMGS_PER_CORE:(c + 1) * IMGS_PER_CORE,
                b * RB:b * RB + nr] = \
                o[:, 0:nr, b].astype(np.float32) * 255.0
    return out.reshape(BATCH, H, W, 1)


# revision 20
# speedup vs baseline: 1.1604x; 1.0086x over previous
"""Adaptive thresholding (11x11 box mean, BORDER_REPLICATE, THRESH_BINARY_INV)
on 8 TRN2 NeuronCores, data-parallel over the batch dim (16 images/core).

v7 design — fused-halo 118-row blocks:
  - Each image is split into 5 row-blocks of [118,118,118,118,40] rows. For
    block b the host packs a [128, 533] fp16 segment: partitions 0-117 the
    block rows, 118-122 the next 5 rows, 123-127 the previous 5 rows
    (BORDER_REPLICATE at the image edges), all scaled y = fp16(x)/4 with an
    11-col zero head and 5-col horizontal replicate margins. Vertical halos
    and edge handling thus live in the DATA, so ONE uniform band matrix
    bmh [128,118] computes the complete vertical sum per block: 2 matmuls
    per block (idn + bmh), 10 per image, 2 LDWEIGHTS groups, and no
    cross-segment PE dependencies.
  - Custom DVE op ADAPT_WSCAN: out = inclusive_scan_add(Src0 - Src1) at
    1 elem/cycle; one scan per image over the flat [128, 5*533] view; the
    duplicated halo partitions are scanned for free (partition-parallel).
  - Block4's dead partitions 40-117 are zeroed on-chip (gpsimd memset) and
    skipped by the DMA (the x plane for block4 is fetched as two partition
    slices).
  - PSUM: tile A [128,3,512] (blocks 0-2, 2 bufs) + tile B [128,2,512]
    (blocks 3-4, 1 buf) = exactly 8 banks. Per image: Sign(PSUM - 60.5)
    as two merged activations (B first so its single buffer recycles
    early) -> fp8e4m3, out at 1 byte/px.
Host: out = (sign >= 0) * 255  (inclusive compare matches x <= mean-2).
"""
import sys
sys.path.insert(0, '/opt/trn_rl_repo')
import numpy as np
import concourse.bass as bass
import concourse.tile as tile
from concourse import bacc, mybir
from concourse.bass_utils import run_bass_kernel_spmd
from concourse import dve_ops as _dops
from concourse.dve_spec import Spec, Src0, Src1, scan, AluOp, lower
from concourse.dve_spec import _has_src1 as _hs1
from concourse.dve_uop import DveOpSpec

F32 = mybir.dt.float32
F16 = mybir.dt.float16
F8 = mybir.dt.float8e4

N_CORES = 8
BATCH, H, W = 128, 512, 512
IMGS_PER_CORE = BATCH // N_CORES      # 16
BLK = 128
RB = 118                              # real rows per block
NSEG = 5                              # blocks per image
RLAST = H - 4 * RB                    # 40 rows in the last block
K = 11
PAD = K // 2                          # 5
ZH = K                                # zero head width
WT = ZH + PAD + W + PAD               # 533 segment width
X0 = ZH + PAD                         # x offset within segment (16)
FLAT = NSEG * WT                      # 2665
SCLEN = FLAT - ZH                     # 2654 scan steps
ROWS = IMGS_PER_CORE * BLK            # 2048 partition-rows per core
OROW = NSEG * RB                      # 590 output rows per image (incl pad)
CN = ("bmh", "bm4", "idn")


def _register_wscan():
    name = "ADAPT_WSCAN"
    if name in _dops._SUB_OPCODE_FOR_NAME:
        return next(o for o in _dops.OPS if o.name == name)
    spec = Spec(
        body=scan(AluOp.ADD, Src0 - Src1),
        reference=lambda in0, in1, s0, s1, imm2: np.cumsum(
            in0.astype(np.float32) - in1.astype(np.float32), axis=-1),
    )
    row = _dops._CUSTOM_DVE_ROW_BASE + len(_dops.OPS)
    _dops._SUB_OPCODE_FOR_NAME[name] = row
    shas = {}
    for ver in ("v3", "v4"):
        tmp = DveOpSpec(name=name, opcode=row, uops=lower(spec, ver=ver),
                        rd1_en=_hs1(spec))
        shas[ver] = tmp.sha(ver)
    op = _dops.DveOp(name, spec, subdim=False, uops_sha=shas)
    _dops.OPS.append(op)
    _dops.CUSTOM_DVE_SPECS[name] = spec
    return op


def _band_matrices(dtype=np.float16):
    # uniform fused band+halo matrix: moving partition k -> output row r
    bmh = np.zeros((BLK, BLK), dtype=dtype)
    for r in range(RB):
        for k in range(RB):
            if abs(r - k) <= PAD:
                bmh[k, r] = 1.0
        for j in range(PAD):
            if r >= RB - PAD + j:     # next rows (partitions 118+j)
                bmh[RB + j, r] = 1.0
            if r <= j:                # prev rows (partitions 123+j)
                bmh[RB + PAD + j, r] = 1.0
    # block4 variant: only RLAST real rows, so the next-row (replicate 511)
    # taps activate at r >= RLAST-PAD+j instead of RB-PAD+j
    bm4 = np.zeros((BLK, BLK), dtype=dtype)
    for r in range(RLAST):
        for k in range(RLAST):
            if abs(r - k) <= PAD:
                bm4[k, r] = 1.0
        for j in range(PAD):
            if r >= RLAST - PAD + j:
                bm4[RLAST + j, r] = 1.0
            if r <= j:
                bm4[RLAST + PAD + j, r] = 1.0
    idn = (-121.0 * np.eye(BLK)).astype(dtype)
    return {"bmh": bmh, "bm4": bm4, "idn": idn}


def _build():
    wop = _register_wscan()
    nc = bacc.Bacc(None, target_bir_lowering=False, debug=False)
    x_d = nc.declare_dram_parameter("x", [ROWS, FLAT], F16, isOutput=False)
    c_d = nc.declare_dram_parameter("consts", [BLK, len(CN) * BLK], F16,
                                    isOutput=False)
    out_d = nc.declare_dram_parameter("out", [ROWS, NSEG * W], F8,
                                      isOutput=True)

    with tile.TileContext(nc) as tc:
        with (
            tc.tile_pool(name="cpool", bufs=1) as cpool,
            tc.tile_pool(name="xin", bufs=4) as x_pool,
            tc.tile_pool(name="scr", bufs=4) as s_pool,
            tc.tile_pool(name="outp", bufs=3) as o_pool,
            tc.tile_pool(name="psA", bufs=2, space=bass.MemorySpace.PSUM) as pa_pool,
            tc.tile_pool(name="psB", bufs=1, space=bass.MemorySpace.PSUM) as pb_pool,
        ):
            cbig = cpool.tile([BLK, len(CN) * BLK], F16, tag="consts")
            nc.scalar.dma_start(cbig[:], c_d[:])
            ct = {nm: cbig[:, j * BLK:(j + 1) * BLK] for j, nm in enumerate(CN)}
            bias_t = cpool.tile([BLK, 1], F32, tag="bias")
            nc.vector.memset(bias_t[:], -242.0 / 4.0)

            imgs = {}
            EDGE = (0, IMGS_PER_CORE - 1)

            def segof(pos):
                return pos * WT + (K - 1)

            def front_img(i):
                ximg = x_pool.tile([BLK, NSEG, WT], F16, tag="ximg")
                r0 = i * BLK
                # block4's dead partitions 50-127 must be finite for the
                # bmh4 matmul (0 x NaN = NaN): zero them once per pool
                # buffer; later tenants never write that region
                if i < 4:
                    nc.gpsimd.memset(ximg[:, NSEG - 1, :], 0.0)
                if i in EDGE:
                    for pos in range(NSEG - 1):
                        nc.sync.dma_start(
                            ximg[:, pos, :],
                            x_d[r0:r0 + BLK, pos * WT:(pos + 1) * WT])
                else:
                    nc.sync.dma_start(
                        ximg[:, 0:NSEG - 1, :],
                        x_d[r0:r0 + BLK, 0:(NSEG - 1) * WT].rearrange(
                            "q (p c) -> q p c", p=NSEG - 1))
                nc.gpsimd.dma_start(
                    ximg[0:RLAST + 2 * PAD, NSEG - 1, :],
                    x_d[r0:r0 + RLAST + 2 * PAD, (NSEG - 1) * WT:FLAT])
                s = s_pool.tile([BLK, SCLEN], F16, tag="scr")
                flat = ximg[:].rearrange("q p c -> q (p c)")
                if i in EDGE:
                    for pos in range(NSEG):
                        o0 = pos * WT
                        nc.vector._custom_dve(
                            wop, out=s[:, o0:o0 + WT - ZH],
                            in0=flat[:, o0 + ZH:o0 + WT],
                            in1=flat[:, o0:o0 + WT - ZH])
                else:
                    nc.vector._custom_dve(
                        wop, out=s[:], in0=flat[:, ZH:FLAT],
                        in1=flat[:, 0:SCLEN])
                imgs[i] = (ximg, s)

            def back_img(i):
                ximg, s = imgs.pop(i)
                psA = pa_pool.tile([BLK, 3, W], F32, tag="psA", name=f"psA_{i}")
                psB = pb_pool.tile([BLK, 2, W], F32, tag="psB", name=f"psB_{i}")
                flat = ximg[:].rearrange("q p c -> q (p c)")
                sseg = [s[:, segof(p):segof(p) + W] for p in range(NSEG)]
                xseg = [flat[:, p * WT + X0:p * WT + X0 + W] for p in range(NSEG)]

                def psof(b):
                    return psA[0:RB, b, :] if b < 3 else psB[0:RB, b - 3, :]

                def mm_idn(b):
                    nc.tensor.matmul(psof(b), ct["idn"][0:RB, 0:RB],
                                     ximg[0:RB, b, X0:X0 + W],
                                     start=True, stop=False)

                def mm_bmh(b):
                    w = ct["bm4"] if b == NSEG - 1 else ct["bmh"]
                    nc.tensor.matmul(psof(b), w[:, 0:RB], sseg[b],
                                     start=False, stop=True)

                # B tile (single buffer) first so its ACT frees it early
                for b in (3, 4):
                    mm_idn(b)
                for b in (3, 4):
                    mm_bmh(b)
                for b in (0, 1, 2):
                    mm_idn(b)
                for b in (0, 1, 2):
                    mm_bmh(b)

                oimg = o_pool.tile([BLK, NSEG, W], F8, tag="oimg")
                orow = out_d[i * BLK:(i + 1) * BLK, :].rearrange(
                    "q (p c) -> q p c", p=NSEG)
                nc.scalar.activation(
                    oimg[0:RB, 3:5, :], psB[0:RB, :, :],
                    mybir.ActivationFunctionType.Sign,
                    bias=bias_t[0:RB], scale=1.0)
                nc.scalar.activation(
                    oimg[0:RB, 0:3, :], psA[0:RB, :, :],
                    mybir.ActivationFunctionType.Sign,
                    bias=bias_t[0:RB], scale=1.0)
                nc.sync.dma_start(orow[0:RB, :, :], oimg[0:RB, :, :])

            front_img(0)
            front_img(1)
            front_img(2)
            for i in range(IMGS_PER_CORE):
                back_img(i)
                if i + 3 < IMGS_PER_CORE:
                    front_img(i + 3)
    nc.compile()
    return nc


_NC_CACHE = None


def _make_in_maps(x: np.ndarray) -> list:
    x = np.asarray(x, dtype=np.float32)
    y = (x.reshape(BATCH, H, W).astype(np.float16) / np.float16(4.0))
    # partition index -> image row per block (with vertical replicate)
    idx = np.zeros((NSEG, BLK), dtype=np.int64)
    live = np.zeros((NSEG, BLK), dtype=bool)
    for b in range(NSEG):
        nr = RB if b < NSEG - 1 else RLAST
        base = b * RB
        idx[b, 0:nr] = base + np.arange(nr)
        live[b, 0:nr] = True
        idx[b, nr:nr + PAD] = np.clip(base + nr + np.arange(PAD), 0, H - 1)
        live[b, nr:nr + PAD] = True
        idx[b, nr + PAD:nr + 2 * PAD] = np.clip(
            base - PAD + np.arange(PAD), 0, H - 1)
        live[b, nr + PAD:nr + 2 * PAD] = True
    planes = y[:, idx, :]                       # [B, NSEG, BLK, W]
    planes[:, ~live, :] = np.float16(0.0)
    plane = np.zeros((BATCH, NSEG, BLK, WT), dtype=np.float16)
    plane[..., X0:X0 + W] = planes
    plane[..., ZH:X0] = planes[..., 0:1]
    plane[..., X0 + W:WT] = planes[..., W - 1:W]
    plane = plane.transpose(0, 2, 1, 3)         # [B, BLK, NSEG, WT]
    cm = _band_matrices()
    cbig = np.concatenate([cm[nm] for nm in CN], axis=1)
    in_maps = []
    for c in range(N_CORES):
        shard = plane[c * IMGS_PER_CORE:(c + 1) * IMGS_PER_CORE].reshape(
            ROWS, FLAT)
        in_maps.append({"x": np.ascontiguousarray(shard),
                        "consts": np.ascontiguousarray(cbig)})
    return in_maps


def kernel(x: np.ndarray) -> np.ndarray:
    global _NC_CACHE
    if _NC_CACHE is None:
        _NC_CACHE = _build()
    nc = _NC_CACHE
    in_maps = _make_in_maps(x)
    res = run_bass_kernel_spmd(nc, in_maps, core_ids=list(range(N_CORES)))
    out = np.empty((BATCH, H, W), dtype=np.float32)
    for c in range(N_CORES):
        sgn = np.asarray(res.results[c]["out"]).view(np.uint8)
        o = (sgn < 0x80).reshape(IMGS_PER_CORE, BLK, NSEG, W)
        for b in range(NSEG):
            nr = RB if b < NSEG - 1 else RLAST
            out[c * IMGS_PER_CORE:(c + 1) * IMGS_PER_CORE,
                b * RB:b * RB + nr] = \
                o[:, 0:nr, b].astype(np.float32) * 255.0
    return out.reshape(BATCH, H, W, 1)


# revision 21
# speedup vs baseline: 2.2009x; 1.8966x over previous
"""Adaptive thresholding (11x11 box mean, BORDER_REPLICATE, THRESH_BINARY_INV)
on 8 TRN2 NeuronCores, data-parallel over the batch dim (16 images/core).

v5 design:
  - Host pre-bakes, per image, a [128, 4x533] fp16 plane holding y = fp16(x)/4
    with an 11-col zero head and 5-col replicate margins per segment. The /4
    scaling halves the fp16 rounding noise of the scan output. No xlo plane:
    fp16(x) quantization keeps rel-err ~1.4e-2 < 2e-2.
  - Custom DVE op ADAPT_WSCAN: out = inclusive_scan_add(Src0 - Src1), a
    single-ALU-stage recurrence at 1 elem/cycle (2x the stock scan, which
    pays a feedback bubble). One scan per image over the flat [128, 2132]
    view; zero heads drain state between segments.
  - PE per image: 14 matmuls into one [128, 4, 512] f32 PSUM tile, ordered
    idn (x-dep only: overlaps the scan) -> bm -> bhn -> bhp, grouped by
    weight matrix. Keeping PE saturated holds it at the fast p-state.
  - ONE merged Scalar activation per image: Sign(PSUM - 60.5) over 4 banks
    -> fp8e4m3 {-1,0,+1}, DMA'd out at 1 byte/px, issued on the scalar
    queue (same engine as the ACT: no cross-engine semaphore).
  - x-in DMA issues on sync (SP); consts as one merged DMA on scalar.
    First/last image: chunked DMA + per-segment scans and matmul order to
    shorten pipeline fill/drain.
Host: out = (sign >= 0) * 255  (inclusive compare matches x <= mean-2).
"""
import sys
sys.path.insert(0, '/opt/trn_rl_repo')
import numpy as np
import concourse.bass as bass
import concourse.tile as tile
from concourse import bacc, mybir
from concourse.bass_utils import run_bass_kernel_spmd
from concourse import dve_ops as _dops
from concourse.dve_spec import Spec, Src0, Src1, scan, AluOp, lower
from concourse.dve_spec import _has_src1 as _hs1
from concourse.dve_uop import DveOpSpec

F32 = mybir.dt.float32
F16 = mybir.dt.float16
F8 = mybir.dt.float8e4

N_CORES = 8
BATCH, H, W = 128, 512, 512
IMGS_PER_CORE = BATCH // N_CORES      # 16
BLK = 128
NBLK = H // BLK                       # 4
K = 11
PAD = K // 2                          # 5
ZH = K                                # zero head width
WT = ZH + PAD + W + PAD               # 533 segment width
X0 = ZH + PAD                         # x offset within segment (16)
FLAT = NBLK * WT                      # 2132
SCLEN = FLAT - ZH                     # 2121 scan steps
ROWS = IMGS_PER_CORE * BLK            # 2048 partition-rows per core
CN = ("bm_top", "bm_mid", "bm_bot", "bhp", "bhn", "idn")


def _register_wscan():
    name = "ADAPT_WSCAN"
    if name in _dops._SUB_OPCODE_FOR_NAME:
        return next(o for o in _dops.OPS if o.name == name)
    spec = Spec(
        body=scan(AluOp.ADD, Src0 - Src1),
        reference=lambda in0, in1, s0, s1, imm2: np.cumsum(
            in0.astype(np.float32) - in1.astype(np.float32), axis=-1),
    )
    row = _dops._CUSTOM_DVE_ROW_BASE + len(_dops.OPS)
    _dops._SUB_OPCODE_FOR_NAME[name] = row
    shas = {}
    for ver in ("v3", "v4"):
        tmp = DveOpSpec(name=name, opcode=row, uops=lower(spec, ver=ver),
                        rd1_en=_hs1(spec))
        shas[ver] = tmp.sha(ver)
    op = _dops.DveOp(name, spec, subdim=False, uops_sha=shas)
    _dops.OPS.append(op)
    _dops.CUSTOM_DVE_SPECS[name] = spec
    return op


def _band_matrices(dtype=np.float16):
    r = np.arange(BLK)
    bm_mid = (np.abs(r[:, None] - r[None, :]) <= PAD).astype(dtype)
    bm_top = bm_mid.copy()
    for rr in range(PAD):
        bm_top[0, rr] += dtype(PAD - rr)
    bm_bot = bm_mid.copy()
    for rr in range(BLK - PAD, BLK):
        bm_bot[BLK - 1, rr] += dtype(rr - (BLK - PAD - 1))
    bhp = np.zeros((BLK, BLK), dtype=dtype)
    for p in range(BLK - PAD, BLK):
        bhp[p, 0:p - (BLK - PAD) + 1] = 1.0
    bhn = np.zeros((BLK, BLK), dtype=dtype)
    for p in range(PAD):
        bhn[p, BLK - PAD + p:BLK] = 1.0
    idn = (-121.0 * np.eye(BLK)).astype(dtype)
    return {"bm_top": bm_top, "bm_mid": bm_mid, "bm_bot": bm_bot,
            "bhp": bhp, "bhn": bhn, "idn": idn}


def _build():
    wop = _register_wscan()
    nc = bacc.Bacc(None, target_bir_lowering=False, debug=False)
    x_d = nc.declare_dram_parameter("x", [ROWS, FLAT], F16, isOutput=False)
    c_d = nc.declare_dram_parameter("consts", [BLK, len(CN) * BLK], F16,
                                    isOutput=False)
    out_d = nc.declare_dram_parameter("out", [ROWS, NBLK * W], F8, isOutput=True)

    with tile.TileContext(nc) as tc:
        with (
            tc.tile_pool(name="cpool", bufs=1) as cpool,
            tc.tile_pool(name="xin", bufs=5) as x_pool,
            tc.tile_pool(name="scr", bufs=5) as s_pool,
            tc.tile_pool(name="outp", bufs=3) as o_pool,
            tc.tile_pool(name="psum", bufs=2, space=bass.MemorySpace.PSUM) as ps_pool,
        ):
            cbig = cpool.tile([BLK, len(CN) * BLK], F16, tag="consts")
            nc.scalar.dma_start(cbig[:], c_d[:])
            ct = {nm: cbig[:, j * BLK:(j + 1) * BLK] for j, nm in enumerate(CN)}
            bias_t = cpool.tile([BLK, 1], F32, tag="bias")
            nc.vector.memset(bias_t[:], -242.0 / 4.0)

            imgs = {}
            EDGE = (0, IMGS_PER_CORE - 1)

            def segof(pos):
                return pos * WT + (K - 1)

            def front_img(i):
                ximg = x_pool.tile([BLK, NBLK, WT], F16, tag="ximg")
                xrow = x_d[i * BLK:(i + 1) * BLK, :].rearrange(
                    "q (p c) -> q p c", p=NBLK)
                s = s_pool.tile([BLK, SCLEN], F16, tag="scr")
                flat = ximg[:].rearrange("q p c -> q (p c)")
                if i in EDGE:
                    # chunked DMA + per-segment scans: finer-grained deps at
                    # the pipeline's fill (i=0) and drain (i=15) ends
                    for pos in range(NBLK):
                        nc.sync.dma_start(ximg[:, pos, :], xrow[:, pos, :])
                    for pos in range(NBLK):
                        o0 = pos * WT
                        nc.vector._custom_dve(
                            wop, out=s[:, o0:o0 + WT - ZH],
                            in0=flat[:, o0 + ZH:o0 + WT],
                            in1=flat[:, o0:o0 + WT - ZH])
                else:
                    nc.sync.dma_start(ximg[:], xrow[:])
                    nc.vector._custom_dve(
                        wop, out=s[:], in0=flat[:, ZH:FLAT],
                        in1=flat[:, 0:SCLEN])
                imgs[i] = (ximg, s)

            def back_img(i):
                ximg, s = imgs.pop(i)
                ps = ps_pool.tile([BLK, NBLK, W], F32, tag="ps", name=f"ps_{i}")
                flat = ximg[:].rearrange("q p c -> q (p c)")

                def mm(wname, bank, mv, start, stop):
                    nc.tensor.matmul(ps[:, bank, :], ct[wname], mv,
                                     start=start, stop=stop)

                bmn = ["bm_top", "bm_mid", "bm_mid", "bm_bot"]
                sseg = [s[:, segof(p):segof(p) + W] for p in range(NBLK)]
                xseg = [flat[:, p * WT + X0:p * WT + X0 + W] for p in range(NBLK)]
                # idn depends only on ximg -> overlaps this image's scan
                for pos in range(NBLK):
                    mm("idn", pos, xseg[pos], True, False)
                if i in EDGE:
                    # per-segment availability order; stops: bank b's last op
                    mm("bm_top", 0, sseg[0], False, False)
                    mm("bhp", 1, sseg[0], False, False)
                    mm("bm_mid", 1, sseg[1], False, False)
                    mm("bhn", 0, sseg[1], False, True)
                    mm("bhp", 2, sseg[1], False, False)
                    mm("bm_mid", 2, sseg[2], False, False)
                    mm("bhn", 1, sseg[2], False, True)
                    mm("bhp", 3, sseg[2], False, False)
                    mm("bm_bot", 3, sseg[3], False, True)
                    mm("bhn", 2, sseg[3], False, True)
                else:
                    for pos in range(NBLK):
                        mm(bmn[pos], pos, sseg[pos], False, False)
                    for pos in range(NBLK - 1):
                        mm("bhn", pos, sseg[pos + 1], False, pos == 0)
                    for pos in range(1, NBLK):
                        mm("bhp", pos, sseg[pos - 1], False, True)
                oimg = o_pool.tile([BLK, NBLK, W], F8, tag="oimg")
                orow = out_d[i * BLK:(i + 1) * BLK, :].rearrange(
                    "q (p c) -> q p c", p=NBLK)
                if i == IMGS_PER_CORE - 1:
                    for pos in range(NBLK):
                        nc.scalar.activation(
                            oimg[:, pos, :], ps[:, pos, :],
                            mybir.ActivationFunctionType.Sign,
                            bias=bias_t[:], scale=1.0)
                        nc.sync.dma_start(orow[:, pos, :], oimg[:, pos, :])
                else:
                    nc.scalar.activation(
                        oimg[:], ps[:], mybir.ActivationFunctionType.Sign,
                        bias=bias_t[:], scale=1.0)
                    nc.scalar.dma_start(orow[:], oimg[:])

            front_img(0)
            front_img(1)
            front_img(2)
            for i in range(IMGS_PER_CORE):
                back_img(i)
                if i + 3 < IMGS_PER_CORE:
                    front_img(i + 3)
    nc.compile()
    return nc


_NC_CACHE = None


def _make_in_maps(x: np.ndarray) -> list:
    x = np.asarray(x, dtype=np.float32)
    y = (x.reshape(BATCH, H, W).astype(np.float16) / np.float16(4.0))
    yq = y.reshape(BATCH, NBLK, BLK, W).transpose(0, 2, 1, 3)
    plane = np.zeros((BATCH, BLK, NBLK, WT), dtype=np.float16)
    plane[..., X0:X0 + W] = yq
    plane[..., ZH:X0] = yq[..., 0:1]
    plane[..., X0 + W:WT] = yq[..., W - 1:W]
    cm = _band_matrices()
    cbig = np.concatenate([cm[nm] for nm in CN], axis=1)
    in_maps = []
    for c in range(N_CORES):
        shard = plane[c * IMGS_PER_CORE:(c + 1) * IMGS_PER_CORE].reshape(
            ROWS, FLAT)
        in_maps.append({"x": np.ascontiguousarray(shard),
                        "consts": np.ascontiguousarray(cbig)})
    return in_maps


def kernel(x: np.ndarray) -> np.ndarray:
    global _NC_CACHE
    if _NC_CACHE is None:
        _NC_CACHE = _build()
    nc = _NC_CACHE
    in_maps = _make_in_maps(x)
    res = run_bass_kernel_spmd(nc, in_maps, core_ids=list(range(N_CORES)))
    out = np.empty((BATCH, H, W), dtype=np.float32)
    for c in range(N_CORES):
        sgn = np.asarray(res.results[c]["out"]).view(np.uint8)
        o = (sgn < 0x80).astype(np.float32) * np.float32(255.0)
        out[c * IMGS_PER_CORE:(c + 1) * IMGS_PER_CORE] = \
            o.reshape(IMGS_PER_CORE, BLK, NBLK, W).transpose(0, 2, 1, 3).reshape(
                IMGS_PER_CORE, H, W)
    return out.reshape(BATCH, H, W, 1)


# revision 22
# speedup vs baseline: 2.2618x; 1.0277x over previous
"""Adaptive thresholding (11x11 box mean, BORDER_REPLICATE, THRESH_BINARY_INV)
on 8 TRN2 NeuronCores, data-parallel over the batch dim (16 images/core).

v5 design:
  - Host pre-bakes, per image, a [128, 4x533] fp16 plane holding y = fp16(x)/4
    with an 11-col zero head and 5-col replicate margins per segment. The /4
    scaling halves the fp16 rounding noise of the scan output. No xlo plane:
    fp16(x) quantization keeps rel-err ~1.4e-2 < 2e-2.
  - Custom DVE op ADAPT_WSCAN: out = inclusive_scan_add(Src0 - Src1), a
    single-ALU-stage recurrence at 1 elem/cycle (2x the stock scan, which
    pays a feedback bubble). One scan per image over the flat [128, 2132]
    view; zero heads drain state between segments.
  - PE per image: 14 matmuls into one [128, 4, 512] f32 PSUM tile, ordered
    idn (x-dep only: overlaps the scan) -> bm -> bhn -> bhp, grouped by
    weight matrix. Keeping PE saturated holds it at the fast p-state.
  - ONE merged Scalar activation per image: Sign(PSUM - 60.5) over 4 banks
    -> fp8e4m3 {-1,0,+1}, DMA'd out at 1 byte/px, issued on the scalar
    queue (same engine as the ACT: no cross-engine semaphore).
  - x-in DMA issues on sync (SP); consts as one merged DMA on scalar.
    First/last image: chunked DMA + per-segment scans and matmul order to
    shorten pipeline fill/drain.
Host: out = (sign >= 0) * 255  (inclusive compare matches x <= mean-2).
"""
import sys
sys.path.insert(0, '/opt/trn_rl_repo')
import numpy as np
import concourse.bass as bass
import concourse.tile as tile
from concourse import bacc, mybir
from concourse.bass_utils import run_bass_kernel_spmd
from concourse import dve_ops as _dops
from concourse.dve_spec import Spec, Src0, Src1, scan, AluOp, lower
from concourse.dve_spec import _has_src1 as _hs1
from concourse.dve_uop import DveOpSpec

F32 = mybir.dt.float32
F16 = mybir.dt.float16
F8 = mybir.dt.float8e4

N_CORES = 8
BATCH, H, W = 128, 512, 512
IMGS_PER_CORE = BATCH // N_CORES      # 16
BLK = 128
NBLK = H // BLK                       # 4
K = 11
PAD = K // 2                          # 5
ZH = K                                # zero head width
WT = ZH + PAD + W + PAD               # 533 segment width
X0 = ZH + PAD                         # x offset within segment (16)
FLAT = NBLK * WT                      # 2132
SCLEN = FLAT - ZH                     # 2121 scan steps
ROWS = IMGS_PER_CORE * BLK            # 2048 partition-rows per core
CN = ("bm_top", "bm_mid", "bm_bot", "bhp", "bhn", "idn")


def _register_wscan():
    name = "ADAPT_WSCAN"
    if name in _dops._SUB_OPCODE_FOR_NAME:
        return next(o for o in _dops.OPS if o.name == name)
    spec = Spec(
        body=scan(AluOp.ADD, Src0 - Src1),
        reference=lambda in0, in1, s0, s1, imm2: np.cumsum(
            in0.astype(np.float32) - in1.astype(np.float32), axis=-1),
    )
    row = _dops._CUSTOM_DVE_ROW_BASE + len(_dops.OPS)
    _dops._SUB_OPCODE_FOR_NAME[name] = row
    shas = {}
    for ver in ("v3", "v4"):
        tmp = DveOpSpec(name=name, opcode=row, uops=lower(spec, ver=ver),
                        rd1_en=_hs1(spec))
        shas[ver] = tmp.sha(ver)
    op = _dops.DveOp(name, spec, subdim=False, uops_sha=shas)
    _dops.OPS.append(op)
    _dops.CUSTOM_DVE_SPECS[name] = spec
    return op


def _band_matrices(dtype=np.float16):
    r = np.arange(BLK)
    bm_mid = (np.abs(r[:, None] - r[None, :]) <= PAD).astype(dtype)
    bm_top = bm_mid.copy()
    for rr in range(PAD):
        bm_top[0, rr] += dtype(PAD - rr)
    bm_bot = bm_mid.copy()
    for rr in range(BLK - PAD, BLK):
        bm_bot[BLK - 1, rr] += dtype(rr - (BLK - PAD - 1))
    bhp = np.zeros((BLK, BLK), dtype=dtype)
    for p in range(BLK - PAD, BLK):
        bhp[p, 0:p - (BLK - PAD) + 1] = 1.0
    bhn = np.zeros((BLK, BLK), dtype=dtype)
    for p in range(PAD):
        bhn[p, BLK - PAD + p:BLK] = 1.0
    idn = (-121.0 * np.eye(BLK)).astype(dtype)
    return {"bm_top": bm_top, "bm_mid": bm_mid, "bm_bot": bm_bot,
            "bhp": bhp, "bhn": bhn, "idn": idn}


def _build():
    wop = _register_wscan()
    nc = bacc.Bacc(None, target_bir_lowering=False, debug=False)
    x_d = nc.declare_dram_parameter("x", [ROWS, FLAT], F16, isOutput=False)
    c_d = nc.declare_dram_parameter("consts", [BLK, len(CN) * BLK], F16,
                                    isOutput=False)
    out_d = nc.declare_dram_parameter("out", [ROWS, NBLK * W], F8, isOutput=True)

    with tile.TileContext(nc) as tc:
        with (
            tc.tile_pool(name="cpool", bufs=1) as cpool,
            tc.tile_pool(name="xin", bufs=5) as x_pool,
            tc.tile_pool(name="scr", bufs=5) as s_pool,
            tc.tile_pool(name="outp", bufs=3) as o_pool,
            tc.tile_pool(name="psum", bufs=2, space=bass.MemorySpace.PSUM) as ps_pool,
        ):
            cbig = cpool.tile([BLK, len(CN) * BLK], F16, tag="consts")
            nc.scalar.dma_start(cbig[:], c_d[:])
            ct = {nm: cbig[:, j * BLK:(j + 1) * BLK] for j, nm in enumerate(CN)}
            bias_t = cpool.tile([BLK, 1], F32, tag="bias")
            nc.vector.memset(bias_t[:], -242.0 / 4.0)

            imgs = {}
            EDGE = (0, IMGS_PER_CORE - 1)

            def segof(pos):
                return pos * WT + (K - 1)

            def front_img(i):
                ximg = x_pool.tile([BLK, NBLK, WT], F16, tag="ximg")
                xrow = x_d[i * BLK:(i + 1) * BLK, :].rearrange(
                    "q (p c) -> q p c", p=NBLK)
                s = s_pool.tile([BLK, SCLEN], F16, tag="scr")
                flat = ximg[:].rearrange("q p c -> q (p c)")
                if i in EDGE:
                    # chunked DMA + per-segment scans: finer-grained deps at
                    # the pipeline's fill (i=0) and drain (i=15) ends
                    for pos in range(NBLK):
                        nc.sync.dma_start(ximg[:, pos, :], xrow[:, pos, :])
                    for pos in range(NBLK):
                        o0 = pos * WT
                        nc.vector._custom_dve(
                            wop, out=s[:, o0:o0 + WT - ZH],
                            in0=flat[:, o0 + ZH:o0 + WT],
                            in1=flat[:, o0:o0 + WT - ZH])
                else:
                    nc.sync.dma_start(ximg[:], xrow[:])
                    nc.vector._custom_dve(
                        wop, out=s[:], in0=flat[:, ZH:FLAT],
                        in1=flat[:, 0:SCLEN])
                imgs[i] = (ximg, s)

            def back_img(i):
                ximg, s = imgs.pop(i)
                ps = ps_pool.tile([BLK, NBLK, W], F32, tag="ps", name=f"ps_{i}")
                flat = ximg[:].rearrange("q p c -> q (p c)")

                def mm(wname, bank, mv, start, stop):
                    nc.tensor.matmul(ps[:, bank, :], ct[wname], mv,
                                     start=start, stop=stop)

                bmn = ["bm_top", "bm_mid", "bm_mid", "bm_bot"]
                sseg = [s[:, segof(p):segof(p) + W] for p in range(NBLK)]
                xseg = [flat[:, p * WT + X0:p * WT + X0 + W] for p in range(NBLK)]
                # idn depends only on ximg -> overlaps this image's scan
                for pos in range(NBLK):
                    mm("idn", pos, xseg[pos], True, False)
                if i in EDGE:
                    # per-segment availability order; stops: bank b's last op
                    mm("bm_top", 0, sseg[0], False, False)
                    mm("bhp", 1, sseg[0], False, False)
                    mm("bm_mid", 1, sseg[1], False, False)
                    mm("bhn", 0, sseg[1], False, True)
                    mm("bhp", 2, sseg[1], False, False)
                    mm("bm_mid", 2, sseg[2], False, False)
                    mm("bhn", 1, sseg[2], False, True)
                    mm("bhp", 3, sseg[2], False, False)
                    mm("bm_bot", 3, sseg[3], False, True)
                    mm("bhn", 2, sseg[3], False, True)
                else:
                    for pos in range(NBLK):
                        mm(bmn[pos], pos, sseg[pos], False, False)
                    for pos in range(NBLK - 1):
                        mm("bhn", pos, sseg[pos + 1], False, pos == 0)
                    for pos in range(1, NBLK):
                        mm("bhp", pos, sseg[pos - 1], False, True)
                oimg = o_pool.tile([BLK, NBLK, W], F8, tag="oimg")
                orow = out_d[i * BLK:(i + 1) * BLK, :].rearrange(
                    "q (p c) -> q p c", p=NBLK)
                nc.scalar.activation(
                    oimg[:], ps[:], mybir.ActivationFunctionType.Sign,
                    bias=bias_t[:], scale=1.0)
                if i == IMGS_PER_CORE - 1:
                    # drain: issue on sync so it does not queue behind the
                    # scalar engine's ACT backlog
                    nc.sync.dma_start(orow[:], oimg[:])
                else:
                    nc.scalar.dma_start(orow[:], oimg[:])

            front_img(0)
            front_img(1)
            front_img(2)
            for i in range(IMGS_PER_CORE):
                back_img(i)
                if i + 3 < IMGS_PER_CORE:
                    front_img(i + 3)
    nc.compile()
    return nc


_NC_CACHE = None


def _make_in_maps(x: np.ndarray) -> list:
    x = np.asarray(x, dtype=np.float32)
    y = (x.reshape(BATCH, H, W).astype(np.float16) / np.float16(4.0))
    yq = y.reshape(BATCH, NBLK, BLK, W).transpose(0, 2, 1, 3)
    plane = np.zeros((BATCH, BLK, NBLK, WT), dtype=np.float16)
    plane[..., X0:X0 + W] = yq
    plane[..., ZH:X0] = yq[..., 0:1]
    plane[..., X0 + W:WT] = yq[..., W - 1:W]
    cm = _band_matrices()
    cbig = np.concatenate([cm[nm] for nm in CN], axis=1)
    in_maps = []
    for c in range(N_CORES):
        shard = plane[c * IMGS_PER_CORE:(c + 1) * IMGS_PER_CORE].reshape(
            ROWS, FLAT)
        in_maps.append({"x": np.ascontiguousarray(shard),
                        "consts": np.ascontiguousarray(cbig)})
    return in_maps


def kernel(x: np.ndarray) -> np.ndarray:
    global _NC_CACHE
    if _NC_CACHE is None:
        _NC_CACHE = _build()
    nc = _NC_CACHE
    in_maps = _make_in_maps(x)
    res = run_bass_kernel_spmd(nc, in_maps, core_ids=list(range(N_CORES)))
    out = np.empty((BATCH, H, W), dtype=np.float32)
    for c in range(N_CORES):
        sgn = np.asarray(res.results[c]["out"]).view(np.uint8)
        o = (sgn < 0x80).astype(np.float32) * np.float32(255.0)
        out[c * IMGS_PER_CORE:(c + 1) * IMGS_PER_CORE] = \
            o.reshape(IMGS_PER_CORE, BLK, NBLK, W).transpose(0, 2, 1, 3).reshape(
                IMGS_PER_CORE, H, W)
    return out.reshape(BATCH, H, W, 1)
